# revision 1
# baseline (speedup 1.0000x reference)
"""Trainium2 Bass kernel for the binarized BasicBlock (dense_cnn).

Contract: kernel(**inputs) takes the FULL unsharded inputs (numpy arrays,
keyed as in reference.setup_inputs()) and returns the FULL output
(32, 128, 56, 56) float32.  Internally shards the batch dim across 8
NeuronCores (pure data parallel, params replicated).

Per-core layout: 4 images as 2 pairs; each pair in 2 half-height units of
28 output rows.  Partitions hold (imgA ch0-63 | imgB ch0-63) for stage-1
tensors.  Design notes (v2):
 - conv1 = 9 shifted matmuls per psum chunk, A/B images on concurrent
   64x64 PE quadrants writing one shared psum tile; the 4*s3 binary-weight
   scale is folded into the bf16 tap weights (exact: bf16(4*s3)*int sums
   stay exact in fp32 psum).
 - residual avgpool on DVE in fp32 (exact, so sign2 never flips); one
   tensor_tensor per chunk merges psum+pool into u.
 - sign ops use a u16 bit-trick on DVE (2x mode) where biases are zero;
   ACT handles a tunable share of sign1 plus the PReLUs, batched to one
   big-N instruction per unit to amortize the ~293ns/inst ACT overhead.
 - stage2: per chunk one [128,1024] psum pair-tile (A bank0 / B bank1),
   matmuls interleaved A/B for quadrant concurrency, one strided ACT
   Prelu over both images.
 - stage2 chunks of unit k-1 are emitted interleaved into unit k's conv
   stream so the PE never idles (stays HAM-warm at 2.4GHz); x slabs
   rotate through 3 buffers so input DMA runs 2-3 units ahead.
"""
import sys

sys.path.insert(0, "/opt/trn_rl_repo")

import numpy as np
import ml_dtypes

import concourse.bacc as bacc
import concourse.mybir as mybir
import concourse.tile as tile
from concourse import bass_utils

# Problem shapes (hardcoded per spec)
B, CIN, H, W = 32, 64, 112, 112
COUT = 2 * CIN
NCORES = 8
BPC = B // NCORES          # images per core = 4
NPAIR = BPC // 2           # image pairs per core = 2
OH, OW = H // 2, W // 2    # 56, 56
HALF = OH // 2             # 28 output rows per unit
NCHUNK = 4                 # psum chunks per unit (7 out rows each)
CROWS = HALF // NCHUNK     # 7
CN = CROWS * OW            # 392 cols per chunk
UN = HALF * OW             # 1568 elems per unit (per partition)
SROWS = 57                 # raw/sign slab rows (input rows 2*oy0-1 .. 2*oy0+55)
SPITCH = 114               # sign slab col pitch (1 left pad + 112 + 1 right pad)
NA_ACT = 34                # sign1 rows handled by ACT (rest on DVE bit-trick)

# param columns
PA1, PB12, PB11, PA2F, PB22F, PS2V, PBS2, PB13, PB23F = range(9)
NPARAM = 9
# weight blocks of 64 cols: conv taps 0..8 (ky*3+kx) pre-scaled by 4*s3;
# then two 128-wide blocks: [wpw1|wpw2] and [diag1|diag2] for M=128
# stage-2 matmuls
NBLK = 9
WCOLS = NBLK * 64 + 256
O_PW = NBLK * 64          # [wpw1|wpw2] at cols O_PW:O_PW+128
O_DIAG = NBLK * 64 + 128  # [diag1|diag2]

_cache = {}


def _build(scal, reps=1):
    """Build the bass program. scal: host-derived scalars/flags.
    reps>1 replicates the whole compute (legacy knob, kept for test.py)."""
    nc = bacc.Bacc("TRN2", target_bir_lowering=False, debug=False)
    f32 = mybir.dt.float32
    bf16 = mybir.dt.bfloat16
    u16 = mybir.dt.uint16
    AF = mybir.ActivationFunctionType
    ALU = mybir.AluOpType

    s3x4 = scal["s3x4"]
    b11_zero = scal["b11_zero"]
    fast_sign2 = scal["fast_sign2"]
    trick_sign2 = scal["trick_sign2"]
    has_b13 = scal["has_b13"]
    has_b23 = scal["has_b23"]

    tc_cm = tile.TileContext(nc)
    tc = tc_cm.__enter__()
    dram_cm = tc.tile_pool(name="dram", bufs=1, space="DRAM")
    dram = dram_cm.__enter__()

    x_d = dram.tile([BPC, CIN, H, W], f32, kind="ExternalInput")
    w_d = dram.tile([128, WCOLS], bf16, kind="ExternalInput")
    p_d = dram.tile([128, NPARAM], f32, kind="ExternalInput")
    y_d = dram.tile([BPC, COUT, OH, OW], f32, kind="ExternalOutput")

    pools = []

    def pool(name, **kw):
        cm = tc.tile_pool(name=name, **kw)
        pools.append(cm)
        return cm.__enter__()

    const = pool("const", bufs=1)
    pers = pool("pers", bufs=1)
    slab = pool("slab", bufs=3)
    work = pool("work", bufs=2)
    work3 = pool("work3", bufs=3)
    psum = pool("psum", bufs=2, space="PSUM")
    psum2 = pool("psum2", bufs=2, space="PSUM")

    wt = const.tile([128, WCOLS], bf16)
    pt = const.tile([128, NPARAM], f32)

    # persistent sign slabs: index by half h (stable pad semantics)
    sp = [pers.tile([128, SROWS * SPITCH], bf16, tag=f"sp{h}", name=f"sp{h}")
          for h in range(2)]
    for h in range(2):
        spv0 = sp[h][:].rearrange("p (r c) -> p r c", r=SROWS)
        nc.vector.memset(spv0[:, 0:1, :], 0.0)
        nc.vector.memset(spv0[:, :, 0:1], 0.0)
        nc.vector.memset(spv0[:, :, 113:114], 0.0)

    def wap(blk):
        return wt[:, 64 * blk:64 * blk + 64]

    units = [(p, h) for _ in range(reps)
             for p in range(NPAIR) for h in range(2)]
    xps = {}
    s4s = {}

    def emit_load(k):
        if k >= len(units) or k in xps:
            return
        p, h = units[k]
        nA = 2 * p
        oy0 = HALF * h
        r0 = 2 * oy0 - 1           # input row of slab row 0
        ld0 = 1 if h == 0 else 0   # first valid slab row
        xp = slab.tile([128, SROWS * W], f32, tag="xp", name=f"xp{k}")
        xpv = xp[:].rearrange("p (r c) -> p r c", r=SROWS)
        # band-split every load: bands align with rowsum slices and conv
        # chunks, so each unit's pre/conv pipeline starts per-band
        bands = [(ld0, 15), (15, 29), (29, 43), (43, SROWS)]
        for (ra, rb) in bands:
            src = x_d[nA:nA + 2, :, r0 + ra:r0 + rb, :].rearrange(
                "i c r w -> (i c) r w")
            nc.sync.dma_start(xpv[:, ra:rb, :], src)
        xps[k] = (xp, bands)

    def emit_pre(k):
        """sign1 -> sp and avgpool -> s4 for unit k."""
        if k >= len(units) or k in s4s:
            return
        p, h = units[k]
        ld0 = 1 if h == 0 else 0
        xp, bands = xps[k]
        xpv = xp[:].rearrange("p (r c) -> p r c", r=SROWS)
        spv = sp[h][:].rearrange("p (r c) -> p r c", r=SROWS)

        prow = work.tile([128, HALF * W], f32, tag="prow", name="prow")
        prv = prow[:].rearrange("p (r c) -> p r c", r=HALF)
        s4 = work3.tile([128, UN], f32, tag="s4", name="s4")
        s4v = s4[:].rearrange("p (r c) -> p r c", r=HALF)

        # per-band pipeline: sign1 (ACT for bands 0-1, DVE u16 bit-trick for
        # bands 2-3 when b11==0), then the band's rowsum slice (GpSimd gets
        # one slice, it is otherwise idle but 2.6x slower than DVE), then a
        # col-sum half per band pair
        for b, (ra, rb) in enumerate(bands):
            if not b11_zero or b < 2:
                kw = {} if b11_zero else {"bias": pt[:, PB11:PB11 + 1]}
                nc.scalar.activation(
                    spv[:, ra:rb, 1:113], xpv[:, ra:rb, :],
                    AF.Sign, **kw)
            else:
                # ACT takes the first rows of band 2 to balance engine load
                # (DVE trick runs at 1x: 1.15ns/el vs ACT 0.93ns/el)
                if b == 2:
                    nc.scalar.activation(
                        spv[:, ra:ra + 4, 1:113], xpv[:, ra:ra + 4, :],
                        AF.Sign)
                    ra = ra + 4
                nr = rb - ra
                xhi = xp[:, ra * W:rb * W].bitcast(u16).rearrange(
                    "p (r c two) -> p r c two", r=nr, two=2)[:, :, :, 1:2]
                nc.vector.tensor_scalar(
                    spv[:, ra:rb, 1:113].bitcast(u16), xhi, 0x8000, 0x3F80,
                    ALU.bitwise_and, ALU.bitwise_or)
            ra2, rb2 = 7 * b, 7 * b + 7
            eng = nc.gpsimd if b == 0 else nc.vector
            eng.tensor_tensor(
                prv[:, ra2:rb2, :],
                xpv[:, 1 + 2 * ra2:1 + 2 * rb2:2, :],
                xpv[:, 2 + 2 * ra2:2 * rb2 + 1:2, :], ALU.add)
            if b % 2 == 1:
                hq = b // 2
                ra3, rb3 = 14 * hq, 14 * hq + 14
                nc.vector.tensor_tensor(
                    s4v[:, ra3:rb3, :], prv[:, ra3:rb3, 0:W:2],
                    prv[:, ra3:rb3, 1:W:2], ALU.add)
        s4s[k] = s4

    # unit 0's bands go out first; wt/pt (needed only at the first matmul,
    # ~6us later) queue behind them instead of ahead
    emit_load(0)
    nc.sync.dma_start(pt[:], p_d[:])
    nc.sync.dma_start(wt[:], w_d[:])
    for k in range(1, 3):
        emit_load(k)

    # HAM warm-up: zero-weight dummy matmuls on band-0 data keep the PE
    # busy from first-band arrival until conv(0), so conv starts at 2.4GHz
    # instead of paying ~3.4us of cold throttle
    zw = const.tile([128, 64], f32)
    nc.vector.memset(zw[:], 0.0)
    xp0 = xps[0][0][:].rearrange("p (r c) -> p r c", r=SROWS)
    dps = psum.tile([128, 1024], f32, tag="ps", name="warmps")
    # a solid >=3.4us PE-busy block right after wt lands (~6us): forces the
    # HAM SHORT window to fire during the fill, so conv(0) runs at 2.4GHz
    # (8 x 427ns bf16 MMs, both operands from wt, garbage psum never read)
    for i in range(8):
        nc.tensor.matmul(
            dps[0:64, 0:512], wt[:, 0:64], wt[:, 64:576],
            start=True, stop=True)
    for i in range(6):
        nc.tensor.matmul(
            dps[0:64, 0:112], zw[:], xp0[:, 1 + i:2 + i, :],
            start=True, stop=True)

    emit_pre(0)

    pending = []  # deferred stage2/prelu2/store emitters from previous unit

    for k, (p, h) in enumerate(units):
        nA, nB = 2 * p, 2 * p + 1
        oy0 = HALF * h
        s4 = s4s.pop(k)
        spv = sp[h][:].rearrange("p (r c) -> p r c", r=SROWS)

        # ---- conv1: 9 taps x 4 chunks, A/B on concurrent 64x64 quads;
        # stage2 chunks of unit k-1 interleave to keep PE dense ----
        # u in bf16: sign2 only needs the sign bit (contiguous u16 view ->
        # packed DVE fast path) and out1 is bf16-rounded downstream anyway
        u = work3.tile([128, UN], bf16, tag="u", name="u")
        out1 = work.tile([128, UN], bf16, tag="out1", name="out1")
        sg2 = work.tile([128, UN], bf16, tag="sg2", name="sg2")
        for half in range(2):
            # chunk-pair psum: chunk 2h at bank 0, chunk 2h+1 at bank 1
            cp = psum.tile([128, 1024], f32, tag="ps", name="ps")
            if half == 0:
                # zero-weight dummy on raw band-0 data: depends only on the
                # load (not sign1), keeps the PE HAM-warm across the unit
                # boundary; tap 0's start=True clears it from the psum
                xpk = xps[k][0][:].rearrange("p (r c) -> p r c", r=SROWS)
                nc.tensor.matmul(
                    cp[0:64, 0:112], zw[:], xpk[:, 1:2, :],
                    start=True, stop=True)
            for cc in range(2):
                c = 2 * half + cc
                for t in range(9):
                    ky, kx = divmod(t, 3)
                    rs = ky + 14 * c
                    for i in range(2):
                        pr = slice(64 * i, 64 * i + 64)
                        rhs = spv[pr, rs:rs + 13:2, kx:kx + 111:2]
                        nc.tensor.matmul(
                            cp[pr, 512 * cc:512 * cc + CN],
                            wap(t)[pr, :], rhs,
                            start=(t == 0), stop=(t == 8),
                        )
                if pending:
                    pending.pop(0)()
            hs = slice(2 * CN * half, 2 * CN * (half + 1))
            cpv = cp[:].rearrange("p (i n) -> p i n", i=2)[:, :, 0:CN]
            uv = u[:, hs].rearrange("p (i n) -> p i n", i=2)
            s4h = s4[:, hs].rearrange("p (i n) -> p i n", i=2)
            nc.vector.scalar_tensor_tensor(
                uv, cpv, s3x4, s4h, ALU.mult, ALU.add)
            # prelu1/sign2 per half-unit: shorter chains into stage2
            nc.scalar.activation(
                out1[:, hs], u[:, hs], AF.Prelu,
                bias=pt[:, PB12:PB12 + 1], scale=0.25,
                alpha=pt[:, PA1:PA1 + 1])
            if trick_sign2:
                nc.vector.tensor_scalar(
                    sg2[:, hs].bitcast(u16), u[:, hs].bitcast(u16),
                    0x8000, 0x3F80,
                    ALU.bitwise_and, ALU.bitwise_or)
            elif fast_sign2:
                nc.scalar.activation(
                    sg2[:, hs], u[:, hs], AF.Sign,
                    bias=pt[:, PB12:PB12 + 1], scale=0.25)
            if half == 0:
                # after unit k's first-half epilogue: next unit's pre-work
                # (ACT sign1 queues behind prelu1(k,h0), not ahead of it)
                emit_pre(k + 1)
        while pending:
            pending.pop(0)()

        if has_b13:
            nc.vector.tensor_scalar(
                out1[:], out1[:], pt[:, PB13:PB13 + 1], None, ALU.add)
        if not fast_sign2 and not trick_sign2:
            nc.scalar.activation(
                sg2[:], out1[:], AF.Sign, bias=pt[:, PBS2:PBS2 + 1])

        emit_load(k + 3)

        # ---- stage 2 (deferred): per chunk a [128,1024] psum pair-tile
        # (A @ cols 0:CN, B @ cols 512:512+CN), matmuls interleaved A/B ----
        stg = work.tile([128, 2 * UN], f32, tag="stg", name="stg")

        def mk_stage2(c, k=k, out1=out1, sg2=sg2, stg=stg,
                      nA=nA, nB=nB, oy0=oy0):
            def emit():
                cs = slice(CN * c, CN * (c + 1))
                p2 = psum2.tile([128, 1024], f32, tag="ps2", name="ps2")
                for i in range(2):
                    pr = slice(64 * i, 64 * i + 64)
                    o2 = slice(512 * i, 512 * i + CN)
                    nc.tensor.matmul(
                        p2[:, o2], wt[pr, O_PW:O_PW + 128], sg2[pr, cs],
                        start=True, stop=False)
                for i in range(2):
                    pr = slice(64 * i, 64 * i + 64)
                    o2 = slice(512 * i, 512 * i + CN)
                    nc.tensor.matmul(
                        p2[:, o2], wt[pr, O_DIAG:O_DIAG + 128], out1[pr, cs],
                        start=False, stop=True)
                # one strided ACT Prelu over both images
                pin = p2[:].rearrange("p (i n) -> p i n", i=2)[:, :, 0:CN]
                pout = stg[:].rearrange("p (i n) -> p i n", i=2)[:, :, cs]
                nc.scalar.activation(
                    pout, pin, AF.Prelu,
                    bias=pt[:, PB22F:PB22F + 1],
                    scale=pt[:, PS2V:PS2V + 1],
                    alpha=pt[:, PA2F:PA2F + 1])
                if has_b23 and c == NCHUNK - 1:
                    nc.vector.tensor_scalar(
                        stg[:], stg[:], pt[:, PB23F:PB23F + 1],
                        None, ALU.add)
                # store per half-unit per image (overlap, good desc size);
                # with b23 the add covers all rows, so store only at the end
                if has_b23:
                    rr = (0, HALF) if c == NCHUNK - 1 else None
                else:
                    rr = {1: (0, 14), NCHUNK - 1: (14, HALF)}.get(c)
                if rr is not None:
                    for i, n in enumerate((nA, nB)):
                        sv = stg[:, UN * i:UN * (i + 1)].rearrange(
                            "p (r c) -> p r c", r=HALF)
                        nc.sync.dma_start(
                            y_d[n, :, oy0 + rr[0]:oy0 + rr[1], :],
                            sv[:, rr[0]:rr[1], :])
            return emit

        pending = [mk_stage2(c) for c in range(NCHUNK)]

    while pending:
        pending.pop(0)()

    for cm in reversed(pools):
        cm.__exit__(None, None, None)
    dram_cm.__exit__(None, None, None)
    tc_cm.__exit__(None, None, None)
    nc.compile()
    return nc, x_d.name, w_d.name, p_d.name, y_d.name


def _prep(inputs):
    f32 = np.float32
    bf = ml_dtypes.bfloat16
    w3 = np.asarray(inputs["w3"], f32)
    wpw1 = np.asarray(inputs["wpw1"], f32)
    wpw2 = np.asarray(inputs["wpw2"], f32)
    a1 = np.asarray(inputs["a1"], f32).reshape(CIN)
    a2 = np.asarray(inputs["a2"], f32).reshape(COUT)
    b11 = np.asarray(inputs["b11"], f32).reshape(CIN)
    b12 = np.asarray(inputs["b12"], f32).reshape(CIN)
    b13 = np.asarray(inputs["b13"], f32).reshape(CIN)
    b21 = np.asarray(inputs["b21"], f32).reshape(CIN)
    b22 = np.asarray(inputs["b22"], f32).reshape(COUT)
    b23 = np.asarray(inputs["b23"], f32).reshape(COUT)

    s3 = float(np.mean(np.abs(w3))) or 1.0
    s1 = float(np.mean(np.abs(wpw1))) or 1.0
    s2 = float(np.mean(np.abs(wpw2))) or 1.0

    # diag entries bf16(1/s_j); prelu2 scale 1/d_j compensates the rounding
    d1 = float(bf(1.0 / s1))
    d2 = float(bf(1.0 / s2))

    whalf = np.zeros((64, WCOLS), f32)
    sgn = np.sign
    for t in range(9):
        ky, kx = divmod(t, 3)
        whalf[:, 64 * t:64 * t + 64] = sgn(w3[:, :, ky, kx]).T
    whalf[:, O_PW:O_PW + 64] = sgn(wpw1[:, :, 0, 0]).T
    whalf[:, O_PW + 64:O_PW + 128] = sgn(wpw2[:, :, 0, 0]).T
    whalf[:, O_DIAG:O_DIAG + 64] = d1 * np.eye(64, dtype=f32)
    whalf[:, O_DIAG + 64:O_DIAG + 128] = d2 * np.eye(64, dtype=f32)
    wfull = np.concatenate([whalf, whalf], axis=0).astype(bf)

    def pairc(v):  # channel vec (64,) -> pair-layout (128,)
        return np.concatenate([v, v])

    params = np.zeros((128, NPARAM), f32)
    params[:, PA1] = pairc(a1)
    params[:, PB12] = pairc(b12)
    params[:, PB11] = pairc(b11)
    params[:, PA2F] = a2
    params[:, PB22F] = b22
    params[:, PS2V] = np.concatenate(
        [np.full(64, 1.0 / d1, f32), np.full(64, 1.0 / d2, f32)])
    params[:, PBS2] = pairc(b13 + b21)
    params[:, PB13] = pairc(b13)
    params[:, PB23F] = b23

    fast_sign2 = bool(np.all(b13 + b21 == 0.0) and np.all(a1 > 0))
    scal = {
        "s3x4": 4.0 * s3,
        "fast_sign2": fast_sign2,
        "trick_sign2": bool(fast_sign2 and np.all(b12 == 0.0)),
        "b11_zero": bool(np.all(b11 == 0.0)),
        "has_b13": bool(np.any(b13 != 0.0)),
        "has_b23": bool(np.any(b23 != 0.0)),
    }
    return wfull, params, scal


def kernel(**inputs):
    x = np.ascontiguousarray(np.asarray(inputs["x"], np.float32))
    wfull, params, scal = _prep(inputs)

    key = tuple(sorted(scal.items())) + (float(params.sum()),)
    if key not in _cache:
        _cache.clear()
        _cache[key] = _build(scal)
    nc, xn, wn, pn, yn = _cache[key]

    in_maps = []
    for i in range(NCORES):
        in_maps.append({
            xn: np.ascontiguousarray(x[BPC * i:BPC * (i + 1)]),
            wn: wfull,
            pn: params,
        })
    res = bass_utils.run_bass_kernel_spmd(nc, in_maps, core_ids=list(range(NCORES)))
    out = np.concatenate([res.results[i][yn] for i in range(NCORES)], axis=0)
    return out.astype(np.float32)



# revision 2
# speedup vs baseline: 1.1839x; 1.1839x over previous
"""Trainium2 Bass kernel for the binarized BasicBlock (dense_cnn) — v3.

Contract: kernel(**inputs) takes the FULL unsharded inputs (numpy arrays,
keyed as in reference.setup_inputs()) and returns the FULL output
(32, 128, 56, 56) float32.  Internally shards the batch dim across 8
NeuronCores (pure data parallel, params replicated).

v3 design (memory-regime: halve HBM traffic, rebalance engines):
 - x is shipped fp16 in a host-packed slab layout [pair, half, 128, 57*112]
   so each unit load is one fully-contiguous [128, 6384] DMA.  Output is
   stored fp16 and upcast on host.  HBM traffic drops 19.3MB -> 9.8MB/core.
 - sign1 runs entirely on DVE as a u16 bit trick (fp16 sign bit -> fp16 +-1),
   eligible for the 4x perf mode (16-bit, step 1, 4B-aligned, single-src).
 - the avgpool shortcut is fused into the conv psum as 4 identity taps
   (weight d0 = fp16(1/(4*s3))) on the raw fp16 x slab; prelu1 reads psum
   directly with scale=s3 (fp32).  No DVE rowsum/colsum/merge at all.
 - conv1 uses 2x2 PE quadrant packing: per tap, 4 concurrent 64x64 matmuls
   (img A/B x chunk c/c+1) with chunk parity swapping the psum partition
   half; all downstream ops are parity-agnostic (params identical per
   partition half) except stage2's rhs slicing, which follows the parity.
 - sign2 = u16 bit trick on fp16 out1 (valid since b13+b21==0 and a1>0).
 - stage2 per chunk: pw + diag matmuls into a [128,1024] psum pair tile,
   one strided Prelu over both images, fp16 stores per half-unit.
"""
import sys

sys.path.insert(0, "/opt/trn_rl_repo")

import numpy as np

import concourse.bacc as bacc
import concourse.mybir as mybir
import concourse.tile as tile
from concourse import bass_utils

# Problem shapes (hardcoded per spec)
B, CIN, H, W = 32, 64, 112, 112
COUT = 2 * CIN
NCORES = 8
BPC = B // NCORES          # images per core = 4
NPAIR = BPC // 2           # image pairs per core = 2
OH, OW = H // 2, W // 2    # 56, 56
HALF = OH // 2             # 28 output rows per unit
NCHUNK = 4                 # psum chunks per unit (7 out rows each)
CROWS = HALF // NCHUNK     # 7
CN = CROWS * OW            # 392 cols per chunk
UN = HALF * OW             # 1568 elems per unit (per partition)
SROWS = 57                 # slab rows (input rows 2*oy0-1 .. 2*oy0+55)
SPITCH = 114               # sign slab col pitch (2 pad cols, signs at 2:114)

# param columns
PA1, PB12, PB11, PA2F, PB22F, PS2V, PBS2, PB13, PB23F = range(9)
NPARAM = 9
# weight blocks of 64 cols: conv taps 0..8 (ky*3+kx), identity d0 block,
# then two 128-wide blocks: [wpw1|wpw2] and [diag1|diag2]
O_ID = 9 * 64              # identity (avgpool) block
O_PW = 10 * 64             # [wpw1|wpw2]
O_DIAG = 10 * 64 + 128     # [diag1|diag2]
WCOLS = 10 * 64 + 256

_cache = {}


def _build(scal, reps=1):
    nc = bacc.Bacc("TRN2", target_bir_lowering=False, debug=False)
    f32 = mybir.dt.float32
    f16 = mybir.dt.float16
    u16 = mybir.dt.uint16
    AF = mybir.ActivationFunctionType
    ALU = mybir.AluOpType

    s3f = scal["s3"]
    b11_zero = scal["b11_zero"]
    trick_sign2 = scal["trick_sign2"]
    has_b13 = scal["has_b13"]
    has_b23 = scal["has_b23"]

    tc_cm = tile.TileContext(nc)
    tc = tc_cm.__enter__()
    dram_cm = tc.tile_pool(name="dram", bufs=1, space="DRAM")
    dram = dram_cm.__enter__()

    x_d = dram.tile([NPAIR, 2, 128, SROWS * W], f16, kind="ExternalInput")
    w_d = dram.tile([128, WCOLS], f16, kind="ExternalInput")
    p_d = dram.tile([128, NPARAM], f32, kind="ExternalInput")
    y_d = dram.tile([BPC, COUT, OH * OW], f16, kind="ExternalOutput")

    pools = []

    def pool(name, **kw):
        cm = tc.tile_pool(name=name, **kw)
        pools.append(cm)
        return cm.__enter__()

    const = pool("const", bufs=1)
    pers = pool("pers", bufs=1)
    slab = pool("slab", bufs=3)
    work = pool("work", bufs=2)
    psum = pool("psum", bufs=2, space="PSUM")
    psum2 = pool("psum2", bufs=2, space="PSUM")

    wt = const.tile([128, WCOLS], f16)
    pt = const.tile([128, NPARAM], f32)

    # persistent sign slabs indexed by half h; cols 0:2 are permanent zero
    # pads (col 1 = input col -1), and for h=0 row 0 is the zero pad row.
    sp = [pers.tile([128, SROWS * SPITCH], f16, tag=f"sp{h}", name=f"sp{h}")
          for h in range(2)]
    for h in range(2):
        spv0 = sp[h][:].rearrange("p (r c) -> p r c", r=SROWS)
        nc.vector.memset(spv0[:, :, 0:2], 0.0)
    nc.vector.memset(
        sp[0][:].rearrange("p (r c) -> p r c", r=SROWS)[:, 0:1, :], 0.0)

    units = [(p, h) for _ in range(reps)
             for p in range(NPAIR) for h in range(2)]
    xps = {}
    signed = set()

    def emit_load(k):
        if k >= len(units) or k in xps:
            return
        p, h = units[k]
        xp = slab.tile([128, SROWS * W], f16, tag="xp", name=f"xp{k}")
        ld0 = 1 if h == 0 else 0
        bands = [(ld0, 15), (15, 29), (29, 43), (43, SROWS)]
        for (ra, rb) in bands:
            nc.sync.dma_start(
                xp[:, ra * W:rb * W], x_d[p, h, :, ra * W:rb * W])
        xps[k] = (xp, bands)

    def emit_sign(k):
        """sign1 bit trick for unit k: fp16 x -> fp16 +-1 in sp[h]."""
        if k >= len(units) or k in signed:
            return
        signed.add(k)
        p, h = units[k]
        xp, bands = xps[k]
        xpv = xp[:].rearrange("p (r c) -> p r c", r=SROWS)
        spv = sp[h][:].rearrange("p (r c) -> p r c", r=SROWS)
        for (ra, rb) in bands:
            if b11_zero:
                nc.vector.tensor_scalar(
                    spv[:, ra:rb, 2:114].bitcast(u16),
                    xpv[:, ra:rb, :].bitcast(u16), 0x8000, 0x3C00,
                    ALU.bitwise_and, ALU.bitwise_or)
            else:
                nc.scalar.activation(
                    spv[:, ra:rb, 2:114], xpv[:, ra:rb, :],
                    AF.Sign, bias=pt[:, PB11:PB11 + 1])

    # queue weights/params first (small), then the first slabs
    nc.sync.dma_start(wt[:], w_d[:])
    nc.sync.dma_start(pt[:], p_d[:])
    for k in range(2):
        emit_load(k)

    # HAM warm-up: a solid >=3.4us PE-busy block as soon as wt lands, so
    # the SHORT window fires during the pipeline fill and the first real
    # conv taps run at 2.4GHz (8 x ~427ns fp16 MMs, garbage psum).
    dps = psum.tile([128, 1024], f32, tag="ps", name="warmps")
    for i in range(8):
        nc.tensor.matmul(
            dps[0:64, 0:512], wt[:, 0:64], wt[:, 64:576],
            start=True, stop=True)

    emit_sign(0)
    emit_load(2)

    def conv_mm(cp_list, spv, xpv, t, q, cc, start, stop):
        """One tap MM for chunk c=2q+cc. Parity cc swaps psum halves."""
        c = 2 * q + cc
        for i in range(2):          # i: img A/B (rhs partition half)
            rp = slice(64 * i, 64 * i + 64)
            ob = 64 * ((i + cc) % 2)  # psum partition half (parity swap)
            op = slice(ob, ob + 64)
            if t < 9:
                ky, kx = divmod(t, 3)
                rhs = spv[rp, ky + 14 * c: ky + 14 * c + 13: 2,
                          1 + kx: 1 + kx + 111: 2]
                w = wt[rp, 64 * t:64 * t + 64]
            else:
                dy, dx = divmod(t - 9, 2)
                rhs = xpv[rp, 1 + 14 * c + dy: 1 + 14 * c + dy + 13: 2,
                          dx: dx + 111: 2]
                w = wt[rp, O_ID:O_ID + 64]
            nc.tensor.matmul(
                cp_list[q][op, 512 * cc:512 * cc + CN], w, rhs,
                start=start, stop=stop)

    pending = []   # deferred stage2 emitters from the previous unit

    for k, (p, h) in enumerate(units):
        nA, nB = 2 * p, 2 * p + 1
        oy0 = HALF * h
        xp, _ = xps[k]
        xpv = xp[:].rearrange("p (r c) -> p r c", r=SROWS)
        spv = sp[h][:].rearrange("p (r c) -> p r c", r=SROWS)

        # ---- conv1 + fused avgpool: 13 taps x (2 chunk-pairs) x 4 quads --
        cp_list = [psum.tile([128, 1024], f32, tag="ps", name=f"ps{k}_{q}")
                   for q in range(2)]
        for t in range(13):
            for q in range(2):
                for cc in range(2):
                    conv_mm(cp_list, spv, xpv, t, q, cc,
                            start=(t == 0), stop=(t == 12))

        # next unit's sign1 on DVE; stage2 of unit k-1 fills the PE gap
        emit_sign(k + 1)
        while pending:
            pending.pop(0)()

        # ---- prelu1 (ACT, psum -> fp16 out1), sign2 (DVE bit trick) ----
        out1 = work.tile([128, UN], f16, tag="out1", name="out1")
        sg2 = work.tile([128, UN], f16, tag="sg2", name="sg2")
        for q in range(2):
            hs = slice(2 * CN * q, 2 * CN * (q + 1))
            pin = cp_list[q][:].rearrange("p (i n) -> p i n", i=2)[:, :, 0:CN]
            pout = out1[:, hs].rearrange("p (i n) -> p i n", i=2)
            nc.scalar.activation(
                pout, pin, AF.Prelu,
                bias=pt[:, PB12:PB12 + 1], scale=s3f,
                alpha=pt[:, PA1:PA1 + 1])
        if has_b13:
            nc.vector.tensor_scalar(
                out1[:], out1[:], pt[:, PB13:PB13 + 1], None, ALU.add)
        if trick_sign2:
            nc.vector.tensor_scalar(
                sg2[:].bitcast(u16), out1[:].bitcast(u16),
                0x8000, 0x3C00, ALU.bitwise_and, ALU.bitwise_or)
        else:
            nc.scalar.activation(
                sg2[:], out1[:], AF.Sign, bias=pt[:, PBS2:PBS2 + 1])

        emit_load(k + 3)

        # ---- stage 2 (deferred into unit k+1's conv window) ----
        stg = work.tile([128, 2 * UN], f16, tag="stg", name="stg")

        def mk_stage2(c, k=k, out1=out1, sg2=sg2, stg=stg,
                      nA=nA, nB=nB, oy0=oy0):
            def emit():
                cs = slice(CN * c, CN * (c + 1))
                p2 = psum2.tile([128, 1024], f32, tag="ps2", name="ps2")
                # slot 0 (cols 0:CN) = img A, slot 1 (512:) = img B;
                # chunk parity decides which sbuf partition half holds A
                for blk, src, st, sp_ in ((O_PW, sg2, True, False),
                                          (O_DIAG, out1, False, True)):
                    for i in range(2):       # i: img A/B (psum slot)
                        rb = 64 * ((i + c) % 2)
                        rp = slice(rb, rb + 64)
                        nc.tensor.matmul(
                            p2[:, 512 * i:512 * i + CN],
                            wt[rp, blk:blk + 128], src[rp, cs],
                            start=st, stop=sp_)
                pin = p2[:].rearrange("p (i n) -> p i n", i=2)[:, :, 0:CN]
                pout = stg[:].rearrange("p (i n) -> p i n", i=2)[:, :, cs]
                nc.scalar.activation(
                    pout, pin, AF.Prelu,
                    bias=pt[:, PB22F:PB22F + 1],
                    scale=pt[:, PS2V:PS2V + 1],
                    alpha=pt[:, PA2F:PA2F + 1])
                if has_b23 and c == NCHUNK - 1:
                    nc.vector.tensor_scalar(
                        stg[:], stg[:], pt[:, PB23F:PB23F + 1],
                        None, ALU.add)
                if has_b23:
                    rr = (0, HALF) if c == NCHUNK - 1 else None
                else:
                    rr = {1: (0, 14), NCHUNK - 1: (14, HALF)}.get(c)
                if rr is not None:
                    for i, n in enumerate((nA, nB)):
                        sv = stg[:, UN * i:UN * (i + 1)].rearrange(
                            "p (r c) -> p r c", r=HALF)
                        nc.sync.dma_start(
                            y_d[n, :, OW * (oy0 + rr[0]):OW * (oy0 + rr[1])],
                            sv[:, rr[0]:rr[1], :])
            return emit

        pending = [mk_stage2(c) for c in range(NCHUNK)]

    while pending:
        pending.pop(0)()

    for cm in reversed(pools):
        cm.__exit__(None, None, None)
    dram_cm.__exit__(None, None, None)
    tc_cm.__exit__(None, None, None)
    nc.compile()
    return nc, x_d.name, w_d.name, p_d.name, y_d.name


def _prep(inputs):
    f32 = np.float32
    f16 = np.float16
    w3 = np.asarray(inputs["w3"], f32)
    wpw1 = np.asarray(inputs["wpw1"], f32)
    wpw2 = np.asarray(inputs["wpw2"], f32)
    a1 = np.asarray(inputs["a1"], f32).reshape(CIN)
    a2 = np.asarray(inputs["a2"], f32).reshape(COUT)
    b11 = np.asarray(inputs["b11"], f32).reshape(CIN)
    b12 = np.asarray(inputs["b12"], f32).reshape(CIN)
    b13 = np.asarray(inputs["b13"], f32).reshape(CIN)
    b21 = np.asarray(inputs["b21"], f32).reshape(CIN)
    b22 = np.asarray(inputs["b22"], f32).reshape(COUT)
    b23 = np.asarray(inputs["b23"], f32).reshape(COUT)

    s3 = f32(np.mean(np.abs(w3))) or f32(1.0)
    s1 = f32(np.mean(np.abs(wpw1))) or f32(1.0)
    s2 = f32(np.mean(np.abs(wpw2))) or f32(1.0)

    d0 = f16(1.0 / (4.0 * float(s3)))
    d1 = f16(1.0 / float(s1))
    d2 = f16(1.0 / float(s2))

    whalf = np.zeros((64, WCOLS), f32)
    sgn = np.sign
    for t in range(9):
        ky, kx = divmod(t, 3)
        whalf[:, 64 * t:64 * t + 64] = sgn(w3[:, :, ky, kx]).T
    whalf[:, O_ID:O_ID + 64] = float(d0) * np.eye(64, dtype=f32)
    whalf[:, O_PW:O_PW + 64] = sgn(wpw1[:, :, 0, 0]).T
    whalf[:, O_PW + 64:O_PW + 128] = sgn(wpw2[:, :, 0, 0]).T
    whalf[:, O_DIAG:O_DIAG + 64] = float(d1) * np.eye(64, dtype=f32)
    whalf[:, O_DIAG + 64:O_DIAG + 128] = float(d2) * np.eye(64, dtype=f32)
    wfull = np.concatenate([whalf, whalf], axis=0).astype(f16)

    def pairc(v):  # channel vec (64,) -> pair-layout (128,)
        return np.concatenate([v, v])

    params = np.zeros((128, NPARAM), f32)
    params[:, PA1] = pairc(a1)
    params[:, PB12] = pairc(b12)
    params[:, PB11] = pairc(b11)
    params[:, PA2F] = a2
    params[:, PB22F] = b22
    params[:, PS2V] = np.concatenate(
        [np.full(64, 1.0 / float(d1), f32), np.full(64, 1.0 / float(d2), f32)])
    params[:, PBS2] = pairc(b13 + b21)
    params[:, PB13] = pairc(b13)
    params[:, PB23F] = b23

    scal = {
        "s3": float(s3),
        "b11_zero": bool(np.all(b11 == 0.0)),
        "trick_sign2": bool(np.all(b13 + b21 == 0.0) and np.all(a1 > 0)),
        "has_b13": bool(np.any(b13 != 0.0)),
        "has_b23": bool(np.any(b23 != 0.0)),
    }
    return wfull, params, scal


def _pack_x(x):
    """x (32,64,112,112) fp32 -> per-core slabs
    [NCORES][NPAIR, 2, 128, 57*112] fp16 (row -1 zero-padded for h=0)."""
    xh = x.astype(np.float16)
    # keep the sign of values that underflow to 0 in fp16 (sign1 must match)
    m = (xh == 0) & (x != 0)
    if m.any():
        xh[m] = np.copysign(np.float16(6e-8), x[m]).astype(np.float16)
    out = np.zeros((NCORES, NPAIR, 2, 2, CIN, SROWS, W), np.float16)
    xc = xh.reshape(NCORES, NPAIR, 2, CIN, H, W)
    for h in range(2):
        r0 = 2 * (HALF * h) - 1
        a = max(r0, 0)
        b = r0 + SROWS
        out[:, :, h, :, :, a - r0:, :] = xc[:, :, :, :, a:b, :]
    # [core, pair, h, img, cin, r, w] -> [core, pair, h, (img cin), r*w]
    return np.ascontiguousarray(
        out.transpose(0, 1, 2, 3, 4, 5, 6)).reshape(
            NCORES, NPAIR, 2, 128, SROWS * W)


def make_in_maps(inputs):
    x = np.asarray(inputs["x"], np.float32)
    wfull, params, scal = _prep(inputs)
    xs = _pack_x(x)
    key = tuple(sorted(scal.items())) + (float(params.sum()),)
    if key not in _cache:
        _cache.clear()
        _cache[key] = _build(scal)
    nc, xn, wn, pn, yn = _cache[key]
    in_maps = [{xn: np.ascontiguousarray(xs[i]), wn: wfull, pn: params}
               for i in range(NCORES)]
    return nc, in_maps, yn


def kernel(**inputs):
    nc, in_maps, yn = make_in_maps(inputs)
    res = bass_utils.run_bass_kernel_spmd(
        nc, in_maps, core_ids=list(range(NCORES)))
    out = np.concatenate(
        [res.results[i][yn].reshape(BPC, COUT, OH, OW) for i in range(NCORES)],
        axis=0)
    return out.astype(np.float32)


# revision 6
# speedup vs baseline: 1.3613x; 1.1498x over previous
"""Trainium2 Bass kernel for the binarized BasicBlock (dense_cnn) — v3.

Contract: kernel(**inputs) takes the FULL unsharded inputs (numpy arrays,
keyed as in reference.setup_inputs()) and returns the FULL output
(32, 128, 56, 56) float32.  Internally shards the batch dim across 8
NeuronCores (pure data parallel, params replicated).

v3 design (memory-regime: halve HBM traffic, rebalance engines):
 - x is shipped fp16 in a host-packed slab layout [pair, half, 128, 57*112]
   so each unit load is one fully-contiguous [128, 6384] DMA.  Output is
   stored fp16 and upcast on host.  HBM traffic drops 19.3MB -> 9.8MB/core.
 - sign1 runs entirely on DVE as a u16 bit trick (fp16 sign bit -> fp16 +-1),
   eligible for the 4x perf mode (16-bit, step 1, 4B-aligned, single-src).
 - the avgpool shortcut is fused into the conv psum as 4 identity taps
   (weight d0 = fp16(1/(4*s3))) on the raw fp16 x slab; prelu1 reads psum
   directly with scale=s3 (fp32).  No DVE rowsum/colsum/merge at all.
 - conv1 uses 2x2 PE quadrant packing: per tap, 4 concurrent 64x64 matmuls
   (img A/B x chunk c/c+1) with chunk parity swapping the psum partition
   half; all downstream ops are parity-agnostic (params identical per
   partition half) except stage2's rhs slicing, which follows the parity.
 - sign2 = u16 bit trick on fp16 out1 (valid since b13+b21==0 and a1>0).
 - stage2 per chunk: pw + diag matmuls into a [128,1024] psum pair tile,
   one strided Prelu over both images, fp16 stores per half-unit.
"""
import sys

sys.path.insert(0, "/opt/trn_rl_repo")

import numpy as np

import concourse.bacc as bacc
import concourse.mybir as mybir
import concourse.tile as tile
from concourse import bass_utils

# Problem shapes (hardcoded per spec)
B, CIN, H, W = 32, 64, 112, 112
COUT = 2 * CIN
NCORES = 8
BPC = B // NCORES          # images per core = 4
NPAIR = BPC // 2           # image pairs per core = 2
OH, OW = H // 2, W // 2    # 56, 56
HALF = OH // 2             # 28 output rows per unit
NCHUNK = 4                 # psum chunks per unit (7 out rows each)
CROWS = HALF // NCHUNK     # 7
CN = CROWS * OW            # 392 cols per chunk
UN = HALF * OW             # 1568 elems per unit (per partition)
SROWS = 57                 # slab rows (input rows 2*oy0-1 .. 2*oy0+55)
SPITCH = 114               # sign slab col pitch (2 pad cols, signs at 2:114)

# param columns
PA1, PB12, PB11, PA2F, PB22F, PS2V, PBS2, PB13, PB23F = range(9)
NPARAM = 9
# weight blocks of 64 cols: conv taps 0..8 (ky*3+kx), identity d0 block,
# then two 128-wide blocks: [wpw1|wpw2] and [diag1|diag2]
O_ID = 9 * 64              # identity (avgpool) block
O_PW = 10 * 64             # [wpw1|wpw2]
O_DIAG = 10 * 64 + 128     # [diag1|diag2]
WCOLS = 10 * 64 + 256

_cache = {}


def _build(scal, reps=1):
    nc = bacc.Bacc("TRN2", target_bir_lowering=False, debug=False)
    f32 = mybir.dt.float32
    f16 = mybir.dt.float16
    u16 = mybir.dt.uint16
    AF = mybir.ActivationFunctionType
    ALU = mybir.AluOpType

    s3f = scal["s3"]
    b11_zero = scal["b11_zero"]
    trick_sign2 = scal["trick_sign2"]
    has_b13 = scal["has_b13"]
    has_b23 = scal["has_b23"]

    tc_cm = tile.TileContext(nc)
    tc = tc_cm.__enter__()
    dram_cm = tc.tile_pool(name="dram", bufs=1, space="DRAM")
    dram = dram_cm.__enter__()

    x_d = dram.tile([NPAIR, 2, 128, SROWS * W], f16, kind="ExternalInput")
    w_d = dram.tile([128, WCOLS], f16, kind="ExternalInput")
    p_d = dram.tile([128, NPARAM], f32, kind="ExternalInput")
    y_d = dram.tile([BPC, COUT, OH * OW], f16, kind="ExternalOutput")

    pools = []

    def pool(name, **kw):
        cm = tc.tile_pool(name=name, **kw)
        pools.append(cm)
        return cm.__enter__()

    const = pool("const", bufs=1)
    pers = pool("pers", bufs=1)
    slab = pool("slab", bufs=4)
    work = pool("work", bufs=2)
    psum = pool("psum", bufs=2, space="PSUM")
    psum2 = pool("psum2", bufs=2, space="PSUM")

    wt = const.tile([128, WCOLS], f16)
    pt = const.tile([128, NPARAM], f32)

    # persistent sign slabs indexed by half h; cols 0:2 are permanent zero
    # pads (col 1 = input col -1), and for h=0 row 0 is the zero pad row.
    sp = [pers.tile([128, SROWS * SPITCH], f16, tag=f"sp{h}", name=f"sp{h}")
          for h in range(2)]
    for h in range(2):
        spv0 = sp[h][:].rearrange("p (r c) -> p r c", r=SROWS)
        nc.vector.memset(spv0[:, :, 0:2], 0.0)
    nc.vector.memset(
        sp[0][:].rearrange("p (r c) -> p r c", r=SROWS)[:, 0:1, :], 0.0)

    units = [(p, h) for _ in range(reps)
             for p in range(NPAIR) for h in range(2)]
    xps = {}
    signed = set()

    def emit_load(k):
        if k >= len(units) or k in xps:
            return
        p, h = units[k]
        xp = slab.tile([128, SROWS * W], f16, tag="xp", name=f"xp{k}")
        ld0 = 1 if h == 0 else 0
        # unit 0 split in two so q0's conv can start early; rest one DMA
        bands = [(ld0, 29), (29, SROWS)] if k == 0 else [(ld0, SROWS)]
        for (ra, rb) in bands:
            nc.sync.dma_start(
                xp[:, ra * W:rb * W], x_d[p, h, :, ra * W:rb * W])
        xps[k] = (xp, bands)

    def emit_sign(k):
        """sign1 bit trick for unit k: fp16 x -> fp16 +-1 in sp[h]."""
        if k >= len(units) or k in signed:
            return
        signed.add(k)
        p, h = units[k]
        xp, bands = xps[k]
        xpv = xp[:].rearrange("p (r c) -> p r c", r=SROWS)
        spv = sp[h][:].rearrange("p (r c) -> p r c", r=SROWS)
        for (ra, rb) in bands:
            if b11_zero:
                nc.vector.tensor_scalar(
                    spv[:, ra:rb, 2:114].bitcast(u16),
                    xpv[:, ra:rb, :].bitcast(u16), 0x8000, 0x3C00,
                    ALU.bitwise_and, ALU.bitwise_or)
            else:
                nc.scalar.activation(
                    spv[:, ra:rb, 2:114], xpv[:, ra:rb, :],
                    AF.Sign, bias=pt[:, PB11:PB11 + 1])

    # all input DMAs issued upfront (slab pool holds all 4 units): the
    # load stream runs dense from t0 and stores slot in behind it
    nc.sync.dma_start(wt[:], w_d[:])
    nc.sync.dma_start(pt[:], p_d[:])
    for k in range(len(units)):
        emit_load(k)

    # HAM warm-up: PE busy from wt arrival until unit 0's first taps
    dps = psum.tile([128, 1024], f32, tag="ps", name="warmps")
    for i in range(5):
        nc.tensor.matmul(
            dps[0:64, 0:512], wt[:, 0:64], wt[:, 64:576],
            start=True, stop=True)

    emit_sign(0)

    def conv_mm(cp, spv, xpv, t, c, cc, start, stop):
        """One tap MM for chunk c. Parity cc swaps psum halves."""
        for i in range(2):          # i: img A/B (rhs partition half)
            rp = slice(64 * i, 64 * i + 64)
            ob = 64 * ((i + cc) % 2)  # psum partition half (parity swap)
            op = slice(ob, ob + 64)
            if t < 9:
                ky, kx = divmod(t, 3)
                rhs = spv[rp, ky + 14 * c: ky + 14 * c + 13: 2,
                          1 + kx: 1 + kx + 111: 2]
                w = wt[rp, 64 * t:64 * t + 64]
            else:
                dy, dx = divmod(t - 9, 2)
                rhs = xpv[rp, 1 + 14 * c + dy: 1 + 14 * c + dy + 13: 2,
                          dx: dx + 111: 2]
                w = wt[rp, O_ID:O_ID + 64]
            nc.tensor.matmul(
                cp[op, 512 * cc:512 * cc + CN], w, rhs,
                start=start, stop=stop)

    pending = []   # deferred stage2 emitters from the previous unit

    for k, (p, h) in enumerate(units):
        nA, nB = 2 * p, 2 * p + 1
        oy0 = HALF * h
        xp, _ = xps[k]
        xpv = xp[:].rearrange("p (r c) -> p r c", r=SROWS)
        spv = sp[h][:].rearrange("p (r c) -> p r c", r=SROWS)

        out1 = work.tile([128, UN], f16, tag="out1", name="out1")
        sg2 = work.tile([128, UN], f16, tag="sg2", name="sg2")
        stg = work.tile([128, 2 * UN], f16, tag="stg", name="stg")

        # ---- conv1 + fused avgpool, q-split: per half-unit 13 taps x
        # 4 concurrent quad MMs; stage2 chunks of unit k-1 interleave at
        # tap boundaries so PE stays dense and prelu2s spread over time --
        for q in range(2):
            cp = psum.tile([128, 1024], f32, tag="ps", name=f"ps{k}_{q}")
            for t in range(13):
                for cc in range(2):
                    conv_mm(cp, spv, xpv, t, 2 * q + cc, cc,
                            start=(t == 0), stop=(t == 12))
                if t in (4, 9) and pending:
                    pending.pop(0)()
            # prelu1 for this half (ACT, psum -> fp16 out1)
            hs = slice(2 * CN * q, 2 * CN * (q + 1))
            pin = cp[:].rearrange("p (i n) -> p i n", i=2)[:, :, 0:CN]
            pout = out1[:, hs].rearrange("p (i n) -> p i n", i=2)
            nc.scalar.activation(
                pout, pin, AF.Prelu,
                bias=pt[:, PB12:PB12 + 1], scale=s3f,
                alpha=pt[:, PA1:PA1 + 1])
            if has_b13:
                nc.vector.tensor_scalar(
                    out1[:, hs], out1[:, hs], pt[:, PB13:PB13 + 1],
                    None, ALU.add)
            if trick_sign2:
                nc.vector.tensor_scalar(
                    sg2[:, hs].bitcast(u16), out1[:, hs].bitcast(u16),
                    0x8000, 0x3C00, ALU.bitwise_and, ALU.bitwise_or)
            else:
                nc.scalar.activation(
                    sg2[:, hs], out1[:, hs], AF.Sign,
                    bias=pt[:, PBS2:PBS2 + 1])
            if q == 0:
                emit_sign(k + 1)

        while pending:
            pending.pop(0)()

        # ---- stage 2 (deferred into unit k+1's conv window) ----
        def mk_stage2(c, k=k, out1=out1, sg2=sg2, stg=stg,
                      nA=nA, nB=nB, oy0=oy0):
            def emit():
                cs = slice(CN * c, CN * (c + 1))
                p2 = psum2.tile([128, 1024], f32, tag="ps2", name="ps2")
                # slot 0 (cols 0:CN) = img A, slot 1 (512:) = img B;
                # chunk parity decides which sbuf partition half holds A
                for blk, src, st, sp_ in ((O_PW, sg2, True, False),
                                          (O_DIAG, out1, False, True)):
                    for i in range(2):       # i: img A/B (psum slot)
                        rb = 64 * ((i + c) % 2)
                        rp = slice(rb, rb + 64)
                        nc.tensor.matmul(
                            p2[:, 512 * i:512 * i + CN],
                            wt[rp, blk:blk + 128], src[rp, cs],
                            start=st, stop=sp_)
                pin = p2[:].rearrange("p (i n) -> p i n", i=2)[:, :, 0:CN]
                pout = stg[:].rearrange("p (i n) -> p i n", i=2)[:, :, cs]
                nc.scalar.activation(
                    pout, pin, AF.Prelu,
                    bias=pt[:, PB22F:PB22F + 1],
                    scale=pt[:, PS2V:PS2V + 1],
                    alpha=pt[:, PA2F:PA2F + 1])
                if has_b23 and c == NCHUNK - 1:
                    nc.vector.tensor_scalar(
                        stg[:], stg[:], pt[:, PB23F:PB23F + 1],
                        None, ALU.add)
                if has_b23:
                    rr = (0, HALF) if c == NCHUNK - 1 else None
                else:
                    rr = {1: (0, 14), NCHUNK - 1: (14, HALF)}.get(c)
                if rr is not None:
                    for i, n in enumerate((nA, nB)):
                        sv = stg[:, UN * i:UN * (i + 1)].rearrange(
                            "p (r c) -> p r c", r=HALF)
                        nc.sync.dma_start(
                            y_d[n, :, OW * (oy0 + rr[0]):OW * (oy0 + rr[1])],
                            sv[:, rr[0]:rr[1], :].rearrange(
                                "p r c -> p (r c)"))
            return emit

        pending = [mk_stage2(c) for c in range(NCHUNK)]

    while pending:
        pending.pop(0)()

    for cm in reversed(pools):
        cm.__exit__(None, None, None)
    dram_cm.__exit__(None, None, None)
    tc_cm.__exit__(None, None, None)
    nc.compile()
    return nc, x_d.name, w_d.name, p_d.name, y_d.name


def _prep(inputs):
    f32 = np.float32
    f16 = np.float16
    w3 = np.asarray(inputs["w3"], f32)
    wpw1 = np.asarray(inputs["wpw1"], f32)
    wpw2 = np.asarray(inputs["wpw2"], f32)
    a1 = np.asarray(inputs["a1"], f32).reshape(CIN)
    a2 = np.asarray(inputs["a2"], f32).reshape(COUT)
    b11 = np.asarray(inputs["b11"], f32).reshape(CIN)
    b12 = np.asarray(inputs["b12"], f32).reshape(CIN)
    b13 = np.asarray(inputs["b13"], f32).reshape(CIN)
    b21 = np.asarray(inputs["b21"], f32).reshape(CIN)
    b22 = np.asarray(inputs["b22"], f32).reshape(COUT)
    b23 = np.asarray(inputs["b23"], f32).reshape(COUT)

    s3 = f32(np.mean(np.abs(w3))) or f32(1.0)
    s1 = f32(np.mean(np.abs(wpw1))) or f32(1.0)
    s2 = f32(np.mean(np.abs(wpw2))) or f32(1.0)

    d0 = f16(1.0 / (4.0 * float(s3)))
    d1 = f16(1.0 / float(s1))
    d2 = f16(1.0 / float(s2))

    whalf = np.zeros((64, WCOLS), f32)
    sgn = np.sign
    for t in range(9):
        ky, kx = divmod(t, 3)
        whalf[:, 64 * t:64 * t + 64] = sgn(w3[:, :, ky, kx]).T
    whalf[:, O_ID:O_ID + 64] = float(d0) * np.eye(64, dtype=f32)
    whalf[:, O_PW:O_PW + 64] = sgn(wpw1[:, :, 0, 0]).T
    whalf[:, O_PW + 64:O_PW + 128] = sgn(wpw2[:, :, 0, 0]).T
    whalf[:, O_DIAG:O_DIAG + 64] = float(d1) * np.eye(64, dtype=f32)
    whalf[:, O_DIAG + 64:O_DIAG + 128] = float(d2) * np.eye(64, dtype=f32)
    wfull = np.concatenate([whalf, whalf], axis=0).astype(f16)

    def pairc(v):  # channel vec (64,) -> pair-layout (128,)
        return np.concatenate([v, v])

    params = np.zeros((128, NPARAM), f32)
    params[:, PA1] = pairc(a1)
    params[:, PB12] = pairc(b12)
    params[:, PB11] = pairc(b11)
    params[:, PA2F] = a2
    params[:, PB22F] = b22
    params[:, PS2V] = np.concatenate(
        [np.full(64, 1.0 / float(d1), f32), np.full(64, 1.0 / float(d2), f32)])
    params[:, PBS2] = pairc(b13 + b21)
    params[:, PB13] = pairc(b13)
    params[:, PB23F] = b23

    scal = {
        "s3": float(s3),
        "b11_zero": bool(np.all(b11 == 0.0)),
        "trick_sign2": bool(np.all(b13 + b21 == 0.0) and np.all(a1 > 0)),
        "has_b13": bool(np.any(b13 != 0.0)),
        "has_b23": bool(np.any(b23 != 0.0)),
    }
    return wfull, params, scal


def _pack_x(x):
    """x (32,64,112,112) fp32 -> per-core slabs
    [NCORES][NPAIR, 2, 128, 57*112] fp16 (row -1 zero-padded for h=0)."""
    xh = x.astype(np.float16)
    # keep the sign of values that underflow to 0 in fp16 (sign1 must match)
    m = (xh == 0) & (x != 0)
    if m.any():
        xh[m] = np.copysign(np.float16(6e-8), x[m]).astype(np.float16)
    out = np.zeros((NCORES, NPAIR, 2, 2, CIN, SROWS, W), np.float16)
    xc = xh.reshape(NCORES, NPAIR, 2, CIN, H, W)
    for h in range(2):
        r0 = 2 * (HALF * h) - 1
        a = max(r0, 0)
        b = r0 + SROWS
        out[:, :, h, :, :, a - r0:, :] = xc[:, :, :, :, a:b, :]
    # [core, pair, h, img, cin, r, w] -> [core, pair, h, (img cin), r*w]
    return np.ascontiguousarray(
        out.transpose(0, 1, 2, 3, 4, 5, 6)).reshape(
            NCORES, NPAIR, 2, 128, SROWS * W)


def make_in_maps(inputs):
    x = np.asarray(inputs["x"], np.float32)
    wfull, params, scal = _prep(inputs)
    xs = _pack_x(x)
    key = tuple(sorted(scal.items())) + (float(params.sum()),)
    if key not in _cache:
        _cache.clear()
        _cache[key] = _build(scal)
    nc, xn, wn, pn, yn = _cache[key]
    in_maps = [{xn: np.ascontiguousarray(xs[i]), wn: wfull, pn: params}
               for i in range(NCORES)]
    return nc, in_maps, yn


def kernel(**inputs):
    nc, in_maps, yn = make_in_maps(inputs)
    res = bass_utils.run_bass_kernel_spmd(
        nc, in_maps, core_ids=list(range(NCORES)))
    out = np.concatenate(
        [res.results[i][yn].reshape(BPC, COUT, OH, OW) for i in range(NCORES)],
        axis=0)
    return out.astype(np.float32)
